# revision 14
# baseline (speedup 1.0000x reference)
"""Trainium2 Bass kernel for weighted-CE + structural-penalty loss (V4).

Full inputs -> data-parallel shard over batch across 8 NeuronCores ->
per-core Bass kernel computes small partial sums -> host combines in
float64.

CE = sum_pos w[t]*(lse - x_t):
  A-side: ACT Exp (interleaved), 3-op DVE tree -> se, ACT Ln -> lse,
    fused scalar_tensor_tensor dot (w[t]*lse) with accum_out.
  B-side: no gather at all -- per class c a single fused
    scalar_tensor_tensor (t==c)*x_c with accum_out reads the fp32
    logits directly (one total pass over x); host applies w_c.

Penalty: per row, pen = pair_sum + P_final - 2*min(0, min_prefix(P)),
  P = cumsum((s==1)-(s==2)) via tensor_tensor_scan (fp32 out -- fp16
  out is 6x slower on HW).  Pair products (shifted mask muls) run on
  the otherwise-idle GPSIMD; their sums are 4x-mode tensor_scalar
  accumulates on DVE.  Rows are split into two 2048-halves on
  partitions r | 64+r (host pre-splits, first half has a 3-column real
  halo, second a zero halo); host chains the halves and adds the one
  clamped boundary term.

Host work is restricted to target/weight-derived prep (O(B*S) int
gathers/casts) and tiny per-core partial combination; every operation
touching logits runs on device.
"""

import numpy as np

import concourse.bass as bass
import concourse.mybir as mybir
import concourse.tile as tile
from concourse import bacc
from concourse.bass_utils import run_bass_kernel_spmd

B, S, C = 512, 4096, 8
PENALTY_WEIGHT = 0.1
NCORES = 8
RB = B // NCORES          # rows (batch) per core
N = RB * S                # positions per core
P = 128                   # SBUF partitions
NP = N // P               # positions per partition (2048)
NCH = 4                   # CE processed in NCH free-dim chunks
PCH = NP // NCH           # positions per partition per chunk (512)
SH = S // 2               # penalty half-row length
HALO = 3
SW = SH + HALO

F32 = mybir.dt.float32
F16 = mybir.dt.float16
OP = mybir.AluOpType
AF = mybir.ActivationFunctionType


def _patch_act_tables():
    """Prefer the single table set containing Exp+Ln+Copy so the kernel
    pays one ACT_TABLE_LOAD instead of alternating per chunk."""
    import concourse.hw_specs as hw_specs
    if getattr(hw_specs, "_loss_kernel_tables_patched", False):
        return
    orig = hw_specs.get_activation_tables

    def patched(arch):
        t = orig(arch)
        pref = "natural_log_exp_and_others"
        if pref not in t:
            return t
        return {k: (v if k == pref else set()) for k, v in t.items()}

    hw_specs.get_activation_tables = patched
    bacc.get_activation_tables = patched
    hw_specs._loss_kernel_tables_patched = True


def build_program(compile=True):
    _patch_act_tables()
    nc = bacc.Bacc("TRN2", target_bir_lowering=False, debug=False)

    logits_d = nc.dram_tensor("logits", [P, NP * C], F32, kind="ExternalInput").ap()
    t_d = nc.dram_tensor("t16", [P, NP], F16, kind="ExternalInput").ap()
    wt_d = nc.dram_tensor("wt16", [P, NP], F16, kind="ExternalInput").ap()
    s_d = nc.dram_tensor("s16", [P, SW], F16, kind="ExternalInput").ap()

    # per chunk: [A-dot, cnt-pad] + 8 class accums for the B side
    ce_d = nc.dram_tensor("ce_acc", [P, NCH, 9], F32, kind="ExternalOutput").ap()
    pen_d = nc.dram_tensor("pen_out", [P, 2], F32, kind="ExternalOutput").ap()
    pair_d = nc.dram_tensor("pair_out", [1, 3, 512], F32, kind="ExternalOutput").ap()

    with tile.TileContext(nc) as tc:
        with (
            tc.tile_pool(name="xb", bufs=2) as xb,
            tc.tile_pool(name="eb", bufs=2) as eb,
            tc.tile_pool(name="tre", bufs=2) as tre,
            tc.tile_pool(name="sml", bufs=2) as sml,
            tc.tile_pool(name="stat", bufs=1) as stat,
            tc.tile_pool(name="pen", bufs=1) as pen,
        ):
            t_sb = stat.tile([P, NP], F16)
            wt_sb = stat.tile([P, NP], F16)
            s_sb = pen.tile([P, SW], F16)
            nc.sync.dma_start(out=t_sb, in_=t_d)
            nc.sync.dma_start(out=wt_sb, in_=wt_d)
            nc.sync.dma_start(out=s_sb, in_=s_d)

            ce_sb = stat.tile([P, NCH, 9], F32)

            # ---------------- CE chunks ----------------
            for k in range(NCH):
                fl = k * PCH * C
                x_t = xb.tile([P, PCH * C], F32, tag="x")
                nc.sync.dma_start(out=x_t, in_=logits_d[:, fl : fl + PCH * C])
                x3 = x_t.rearrange("p (n c) -> p n c", c=C)
                tk = t_sb[:, k * PCH : (k + 1) * PCH]
                wk = wt_sb[:, k * PCH : (k + 1) * PCH]

                # B side: S_c = sum (t==c) * x_c, fused, straight off fp32 x
                jb = sml.tile([P, PCH], F16, tag="jb")
                for c in range(C):
                    nc.vector.scalar_tensor_tensor(
                        out=jb, in0=tk, scalar=float(c),
                        in1=x3[:, :, c], op0=OP.is_equal, op1=OP.mult,
                        accum_out=ce_sb[:, k, 1 + c : 2 + c])

                # A side: lse then fused dot with w[t]
                e_t = eb.tile([P, PCH, C], F16, tag="e")
                nc.scalar.activation(e_t, x3, AF.Exp)
                s4 = tre.tile([P, PCH, 4], F16, tag="s4")
                nc.vector.tensor_add(s4, e_t[:, :, 0:4], e_t[:, :, 4:8])
                s2 = tre.tile([P, PCH, 2], F16, tag="s2")
                nc.vector.tensor_add(s2, s4[:, :, 0:2], s4[:, :, 2:4])
                se = sml.tile([P, PCH], F16, tag="se")
                se3 = se.rearrange("p (n o) -> p n o", o=1)
                nc.vector.tensor_add(se3, s2[:, :, 0:1], s2[:, :, 1:2])
                lse = sml.tile([P, PCH], F16, tag="lse")
                nc.scalar.activation(lse, se, AF.Ln)
                ja = sml.tile([P, PCH], F16, tag="ja")
                nc.vector.scalar_tensor_tensor(
                    out=ja, in0=wk, scalar=1.0, in1=lse,
                    op0=OP.mult, op1=OP.mult,
                    accum_out=ce_sb[:, k, 0:1])

            nc.sync.dma_start(out=ce_d, in_=ce_sb)

            # ---------------- penalty ----------------
            lp_t = pen.tile([P, SW], F16)
            rp_t = pen.tile([P, SW], F16)
            ee_t = pen.tile([P, SW], F16)
            nc.vector.tensor_scalar(out=lp_t, in0=s_sb, scalar1=1.0,
                                    scalar2=None, op0=OP.is_equal)
            nc.vector.tensor_scalar(out=rp_t, in0=s_sb, scalar1=2.0,
                                    scalar2=None, op0=OP.is_equal)
            nc.vector.tensor_scalar(out=ee_t, in0=s_sb, scalar1=3.0,
                                    scalar2=None, op0=OP.is_equal)

            # shifted pair products on GPSIMD (otherwise idle)
            er_t = pen.tile([P, SW], F16)
            eer_t = pen.tile([P, SW], F16)
            prs = pen.tile([P, 3, SH], F16)
            nc.gpsimd.tensor_tensor(out=er_t[:, 0 : SW - 1],
                                    in0=ee_t[:, 0 : SW - 1],
                                    in1=rp_t[:, 1:SW], op=OP.mult)
            nc.gpsimd.tensor_tensor(out=eer_t[:, 0 : SW - 2],
                                    in0=ee_t[:, 0 : SW - 2],
                                    in1=er_t[:, 1 : SW - 1], op=OP.mult)
            nc.gpsimd.tensor_tensor(out=prs[:, 0, :], in0=lp_t[:, 0:SH],
                                    in1=rp_t[:, 1 : SH + 1], op=OP.mult)
            nc.gpsimd.tensor_tensor(out=prs[:, 1, :], in0=lp_t[:, 0:SH],
                                    in1=er_t[:, 1 : SH + 1], op=OP.mult)
            nc.gpsimd.tensor_tensor(out=prs[:, 2, :], in0=lp_t[:, 0:SH],
                                    in1=eer_t[:, 1 : SH + 1], op=OP.mult)

            # pair sums on the idle tensor engine: ones-matmul column
            # reduction into one PSUM bank per pair kind; host finishes.
            with tc.tile_pool(name="ps", bufs=1, space="PSUM") as psp:
                ones_t = stat.tile([P, 1], F16)
                nc.vector.memset(ones_t, 1.0)
                ps_pair = psp.tile([1, 3, 512], F32, name="ps_pair")
                nmm = SH // 512
                for i in range(3):
                    for wmm in range(nmm):
                        nc.tensor.matmul(
                            ps_pair[:, i, :], lhsT=ones_t,
                            rhs=prs[:, i, wmm * 512 : (wmm + 1) * 512],
                            start=(wmm == 0), stop=(wmm == nmm - 1))
                pair_sb = stat.tile([1, 3, 512], F32)
                nc.scalar.activation(pair_sb, ps_pair, AF.Copy)
                nc.sync.dma_start(out=pair_d, in_=pair_sb)

            pen_sb = stat.tile([P, 2], F32)

            # P = cumsum(lp - rp); need P_final and min-prefix (fp32 out --
            # fp16-out scan measured 6x slower)
            p_t = pen.tile([P, SH], F32)
            nc.vector.tensor_tensor_scan(out=p_t, data0=lp_t[:, 0:SH],
                                         data1=rp_t[:, 0:SH], initial=0.0,
                                         op0=OP.add, op1=OP.subtract)
            nc.vector.tensor_copy(out=pen_sb[:, 0:1], in_=p_t[:, SH - 1 : SH])
            nc.vector.tensor_reduce(out=pen_sb[:, 1:2], in_=p_t,
                                    axis=mybir.AxisListType.X, op=OP.min)
            nc.sync.dma_start(out=pen_d, in_=pen_sb)

    if compile:
        nc.compile()
    return nc


_program = None


def _get_program():
    global _program
    if _program is None:
        _program = build_program()
    return _program


def _pair_boundary(s):
    """The only clamped boundary pair term not covered on device:
    4 * [s[S-3]==1][s[S-2]==3][s[S-1]==2] per row."""
    m = (s[:, -3] == 1) & (s[:, -2] == 3) & (s[:, -1] == 2)
    return 4.0 * float(m.sum())


def make_in_maps(logits, targets, predicted_structures, ce_weights):
    lg = np.ascontiguousarray(logits, dtype=np.float32)
    t = np.asarray(targets)
    s = np.asarray(predicted_structures).reshape(B, S)
    t16 = t.astype(np.float16)
    wt16 = np.asarray(ce_weights, np.float32)[t.astype(np.int64)].astype(np.float16)
    s16 = s.astype(np.float16)
    in_maps = []
    for core in range(NCORES):
        rows = slice(core * RB, (core + 1) * RB)
        sc = s16[rows]                      # [RB, S]
        sh = np.zeros((P, SW), np.float16)
        sh[0:RB, :] = sc[:, 0:SW]           # first half + real halo
        sh[RB:P, 0:SH] = sc[:, SH:S]        # second half, zero halo
        in_maps.append({
            "logits": lg[rows].reshape(P, NP * C),
            "t16": np.ascontiguousarray(t16[rows].reshape(P, NP)),
            "wt16": np.ascontiguousarray(wt16[rows].reshape(P, NP)),
            "s16": sh,
        })
    return in_maps, s, t


def combine_partials(results, s_full, targets, ce_weights):
    """Host-side (float64) combination of per-core device partials."""
    w = np.asarray(ce_weights, np.float64)
    ce_num = 0.0
    pen = 0.0
    for r in results:
        ce = r["ce_acc"].astype(np.float64)         # [P, NCH, 9]
        ce_num += ce[:, :, 0].sum()                 # sum w[t]*lse
        ce_num -= (ce[:, :, 1:9].sum((0, 1)) * w).sum()   # sum w_c * S_c
        po = r["pen_out"].astype(np.float64)        # [P, 2]
        pfa, mpa = po[0:RB, 0], po[0:RB, 1]
        pfb, mpb = po[RB:P, 0], po[RB:P, 1]
        pf = pfa + pfb
        mp = np.minimum(mpa, pfa + mpb)
        pen += (pf - 2.0 * np.minimum(0.0, mp)).sum()
        pa = r["pair_out"].astype(np.float64).reshape(3, 512)
        pen += (pa.sum(1) * np.array([2.0, 3.0, 4.0])).sum()
    pen += _pair_boundary(s_full)
    ce_loss = ce_num / (B * S)
    nnz = float((targets != 0).sum())
    penalty = pen / nnz
    return np.float32(ce_loss + PENALTY_WEIGHT * penalty)


def kernel(logits, targets, predicted_structures, ce_weights):
    in_maps, s, t = make_in_maps(logits, targets, predicted_structures,
                                 ce_weights)
    nc = _get_program()
    res = run_bass_kernel_spmd(nc, in_maps, core_ids=list(range(NCORES)))
    return combine_partials(res.results, s, t, ce_weights)


# revision 18
# speedup vs baseline: 1.1545x; 1.1545x over previous
"""Trainium2 Bass kernel for weighted-CE + structural-penalty loss (V4).

Full inputs -> data-parallel shard over batch across 8 NeuronCores ->
per-core Bass kernel computes small partial sums -> host combines in
float64.

CE = sum_pos w[t]*(lse - x_t):
  A-side: ACT Exp (interleaved), 3-op DVE tree -> se, ACT Ln -> lse,
    fused scalar_tensor_tensor dot (w[t]*lse) with accum_out.
  B-side: no gather at all -- per class c a single fused
    scalar_tensor_tensor (t==c)*x_c with accum_out reads the fp32
    logits directly (one total pass over x); host applies w_c.

Penalty: per row, pen = pair_sum + P_final - 2*min(0, min_prefix(P)),
  P = cumsum((s==1)-(s==2)) via tensor_tensor_scan (fp32 out -- fp16
  out is 6x slower on HW).  Pair products (shifted mask muls) run on
  the otherwise-idle GPSIMD; their sums are 4x-mode tensor_scalar
  accumulates on DVE.  Rows are split into two 2048-halves on
  partitions r | 64+r (host pre-splits, first half has a 3-column real
  halo, second a zero halo); host chains the halves and adds the one
  clamped boundary term.

Host work is restricted to target/weight-derived prep (O(B*S) int
gathers/casts) and tiny per-core partial combination; every operation
touching logits runs on device.
"""

import numpy as np

import concourse.bass as bass
import concourse.mybir as mybir
import concourse.tile as tile
from concourse import bacc
from concourse.bass_utils import run_bass_kernel_spmd

B, S, C = 512, 4096, 8
PENALTY_WEIGHT = 0.1
NCORES = 8
RB = B // NCORES          # rows (batch) per core
N = RB * S                # positions per core
P = 128                   # SBUF partitions
NP = N // P               # positions per partition (2048)
NCH = 4                   # CE processed in NCH free-dim chunks
PCH = NP // NCH           # positions per partition per chunk (512)
SH = S // 2               # penalty half-row length
HALO = 3
SW = SH + HALO

F32 = mybir.dt.float32
F16 = mybir.dt.float16
OP = mybir.AluOpType
AF = mybir.ActivationFunctionType


def _patch_act_tables():
    """Prefer the single table set containing Exp+Ln+Copy so the kernel
    pays one ACT_TABLE_LOAD instead of alternating per chunk."""
    import concourse.hw_specs as hw_specs
    if getattr(hw_specs, "_loss_kernel_tables_patched", False):
        return
    orig = hw_specs.get_activation_tables

    def patched(arch):
        t = orig(arch)
        pref = "natural_log_exp_and_others"
        if pref not in t:
            return t
        return {k: (v if k == pref else set()) for k, v in t.items()}

    hw_specs.get_activation_tables = patched
    bacc.get_activation_tables = patched
    hw_specs._loss_kernel_tables_patched = True


def build_program(compile=True):
    _patch_act_tables()
    nc = bacc.Bacc("TRN2", target_bir_lowering=False, debug=False)

    logits_d = nc.dram_tensor("logits", [P, NP * C], F32, kind="ExternalInput").ap()
    t_d = nc.dram_tensor("t16", [P, NP], F16, kind="ExternalInput").ap()
    wt_d = nc.dram_tensor("wt16", [P, NP], F16, kind="ExternalInput").ap()
    s_d = nc.dram_tensor("s16", [P, SW], F16, kind="ExternalInput").ap()

    ce_d = nc.dram_tensor("ce_acc", [P, NCH], F32, kind="ExternalOutput").ap()
    dx_d = nc.dram_tensor("diag_x", [P, 8, P], F32, kind="ExternalOutput").ap()
    pen_d = nc.dram_tensor("pen_out", [P, 2], F32, kind="ExternalOutput").ap()
    pair_d = nc.dram_tensor("pair_out", [1, 3, 512], F32, kind="ExternalOutput").ap()

    with tile.TileContext(nc) as tc:
        with (
            tc.tile_pool(name="xb", bufs=2) as xb,
            tc.tile_pool(name="eb", bufs=2) as eb,
            tc.tile_pool(name="tre", bufs=2) as tre,
            tc.tile_pool(name="sml", bufs=2) as sml,
            tc.tile_pool(name="stat", bufs=1) as stat,
            tc.tile_pool(name="pen", bufs=1) as pen,
        ):
            t_sb = stat.tile([P, NP], F16)
            wt_sb = stat.tile([P, NP], F16)
            s_sb = pen.tile([P, SW], F16)
            nc.sync.dma_start(out=t_sb, in_=t_d)
            nc.sync.dma_start(out=wt_sb, in_=wt_d)
            nc.sync.dma_start(out=s_sb, in_=s_d)

            ce_sb = stat.tile([P, NCH], F32)

            with tc.tile_pool(name="psx", bufs=1, space="PSUM") as psx:
                ps_x = [psx.tile([P, 4, P], F32, name=f"ps_x{q}")
                        for q in range(2)]

                # ---------------- CE chunks ----------------
                for k in range(NCH):
                    fl = k * PCH * C
                    x_t = xb.tile([P, PCH * C], F32, tag="x")
                    nc.sync.dma_start(out=x_t, in_=logits_d[:, fl : fl + PCH * C])
                    x3 = x_t.rearrange("p (n c) -> p n c", c=C)
                    tk = t_sb[:, k * PCH : (k + 1) * PCH]
                    wk = wt_sb[:, k * PCH : (k + 1) * PCH]

                    # B side on TensorE: per-class diagonal matmuls
                    # accumulate S_c partials; masks @4x, xh cast on ACT.
                    m2 = eb.tile([P, C, PCH], F16, tag="m2")
                    for c in range(C):
                        nc.vector.tensor_scalar(out=m2[:, c, :], in0=tk,
                                                scalar1=float(c), scalar2=None,
                                                op0=OP.is_equal)
                    xh = eb.tile([P, PCH, C], F16, tag="xh")
                    nc.scalar.activation(xh, x3, AF.Copy)
                    last = k == NCH - 1
                    for c in range(C):
                        q, sl = divmod(c, 4)
                        for b in range(PCH // P):
                            bs = slice(b * P, (b + 1) * P)
                            nc.tensor.matmul(
                                ps_x[q][:, sl, :], lhsT=m2[:, c, bs],
                                rhs=xh[:, bs, c],
                                start=(k == 0 and b == 0 and c in (0, 4)),
                                stop=(last and c in (3, 7) and b == PCH // P - 1))

                    # A side: lse then fused dot with w[t]
                    e_t = eb.tile([P, PCH, C], F16, tag="e")
                    nc.scalar.activation(e_t, x3, AF.Exp)
                    s4 = tre.tile([P, PCH, 4], F16, tag="s4")
                    nc.vector.tensor_add(s4, e_t[:, :, 0:4], e_t[:, :, 4:8])
                    s2 = tre.tile([P, PCH, 2], F16, tag="s2")
                    nc.vector.tensor_add(s2, s4[:, :, 0:2], s4[:, :, 2:4])
                    se = sml.tile([P, PCH], F16, tag="se")
                    se3 = se.rearrange("p (n o) -> p n o", o=1)
                    nc.vector.tensor_add(se3, s2[:, :, 0:1], s2[:, :, 1:2])
                    lse = sml.tile([P, PCH], F16, tag="lse")
                    nc.scalar.activation(lse, se, AF.Ln)
                    ja = sml.tile([P, PCH], F16, tag="ja")
                    nc.vector.scalar_tensor_tensor(
                        out=ja, in0=wk, scalar=1.0, in1=lse,
                        op0=OP.mult, op1=OP.mult,
                        accum_out=ce_sb[:, k : k + 1])

                nc.sync.dma_start(out=ce_d, in_=ce_sb)
                dx_sb = stat.tile([P, 8, P], F32)
                for q in range(2):
                    nc.scalar.activation(dx_sb[:, q * 4 : (q + 1) * 4, :],
                                         ps_x[q][:, :, :], AF.Copy)
                nc.sync.dma_start(out=dx_d, in_=dx_sb)

            # ---------------- penalty ----------------
            lp_t = pen.tile([P, SW], F16)
            rp_t = pen.tile([P, SW], F16)
            ee_t = pen.tile([P, SW], F16)
            nc.vector.tensor_scalar(out=lp_t, in0=s_sb, scalar1=1.0,
                                    scalar2=None, op0=OP.is_equal)
            nc.vector.tensor_scalar(out=rp_t, in0=s_sb, scalar1=2.0,
                                    scalar2=None, op0=OP.is_equal)
            nc.vector.tensor_scalar(out=ee_t, in0=s_sb, scalar1=3.0,
                                    scalar2=None, op0=OP.is_equal)

            # shifted pair products on GPSIMD (otherwise idle)
            er_t = pen.tile([P, SW], F16)
            eer_t = pen.tile([P, SW], F16)
            prs = pen.tile([P, 3, SH], F16)
            nc.gpsimd.tensor_tensor(out=er_t[:, 0 : SW - 1],
                                    in0=ee_t[:, 0 : SW - 1],
                                    in1=rp_t[:, 1:SW], op=OP.mult)
            nc.gpsimd.tensor_tensor(out=eer_t[:, 0 : SW - 2],
                                    in0=ee_t[:, 0 : SW - 2],
                                    in1=er_t[:, 1 : SW - 1], op=OP.mult)
            nc.gpsimd.tensor_tensor(out=prs[:, 0, :], in0=lp_t[:, 0:SH],
                                    in1=rp_t[:, 1 : SH + 1], op=OP.mult)
            nc.gpsimd.tensor_tensor(out=prs[:, 1, :], in0=lp_t[:, 0:SH],
                                    in1=er_t[:, 1 : SH + 1], op=OP.mult)
            nc.gpsimd.tensor_tensor(out=prs[:, 2, :], in0=lp_t[:, 0:SH],
                                    in1=eer_t[:, 1 : SH + 1], op=OP.mult)

            # pair sums on the idle tensor engine: ones-matmul column
            # reduction into one PSUM bank per pair kind; host finishes.
            with tc.tile_pool(name="ps", bufs=1, space="PSUM") as psp:
                ones_t = stat.tile([P, 1], F16)
                nc.vector.memset(ones_t, 1.0)
                ps_pair = psp.tile([1, 3, 512], F32, name="ps_pair")
                nmm = SH // 512
                for i in range(3):
                    for wmm in range(nmm):
                        nc.tensor.matmul(
                            ps_pair[:, i, :], lhsT=ones_t,
                            rhs=prs[:, i, wmm * 512 : (wmm + 1) * 512],
                            start=(wmm == 0), stop=(wmm == nmm - 1))
                pair_sb = stat.tile([1, 3, 512], F32)
                nc.scalar.activation(pair_sb, ps_pair, AF.Copy)
                nc.sync.dma_start(out=pair_d, in_=pair_sb)

            pen_sb = stat.tile([P, 2], F32)

            # P = cumsum(lp - rp); need P_final and min-prefix (fp32 out --
            # fp16-out scan measured 6x slower)
            p_t = pen.tile([P, SH], F32)
            nc.vector.tensor_tensor_scan(out=p_t, data0=lp_t[:, 0:SH],
                                         data1=rp_t[:, 0:SH], initial=0.0,
                                         op0=OP.add, op1=OP.subtract)
            nc.vector.tensor_copy(out=pen_sb[:, 0:1], in_=p_t[:, SH - 1 : SH])
            nc.vector.tensor_reduce(out=pen_sb[:, 1:2], in_=p_t,
                                    axis=mybir.AxisListType.X, op=OP.min)
            nc.sync.dma_start(out=pen_d, in_=pen_sb)

    if compile:
        nc.compile()
    return nc


_program = None


def _get_program():
    global _program
    if _program is None:
        _program = build_program()
    return _program


def _pair_boundary(s):
    """The only clamped boundary pair term not covered on device:
    4 * [s[S-3]==1][s[S-2]==3][s[S-1]==2] per row."""
    m = (s[:, -3] == 1) & (s[:, -2] == 3) & (s[:, -1] == 2)
    return 4.0 * float(m.sum())


def make_in_maps(logits, targets, predicted_structures, ce_weights):
    lg = np.ascontiguousarray(logits, dtype=np.float32)
    t = np.asarray(targets)
    s = np.asarray(predicted_structures).reshape(B, S)
    t16 = t.astype(np.float16)
    wt16 = np.asarray(ce_weights, np.float32)[t.astype(np.int64)].astype(np.float16)
    s16 = s.astype(np.float16)
    in_maps = []
    for core in range(NCORES):
        rows = slice(core * RB, (core + 1) * RB)
        sc = s16[rows]                      # [RB, S]
        sh = np.zeros((P, SW), np.float16)
        sh[0:RB, :] = sc[:, 0:SW]           # first half + real halo
        sh[RB:P, 0:SH] = sc[:, SH:S]        # second half, zero halo
        in_maps.append({
            "logits": lg[rows].reshape(P, NP * C),
            "t16": np.ascontiguousarray(t16[rows].reshape(P, NP)),
            "wt16": np.ascontiguousarray(wt16[rows].reshape(P, NP)),
            "s16": sh,
        })
    return in_maps, s, t


def combine_partials(results, s_full, targets, ce_weights):
    """Host-side (float64) combination of per-core device partials."""
    w = np.asarray(ce_weights, np.float64)
    ce_num = 0.0
    pen = 0.0
    p_idx = np.arange(P)
    for r in results:
        ce_num += r["ce_acc"].astype(np.float64).sum()     # sum w[t]*lse
        dx = r["diag_x"].astype(np.float64)                # [P, 8, P]
        Sc = dx[p_idx, :, p_idx].sum(0)
        ce_num -= (Sc * w).sum()                           # sum w_c * S_c
        po = r["pen_out"].astype(np.float64)        # [P, 2]
        pfa, mpa = po[0:RB, 0], po[0:RB, 1]
        pfb, mpb = po[RB:P, 0], po[RB:P, 1]
        pf = pfa + pfb
        mp = np.minimum(mpa, pfa + mpb)
        pen += (pf - 2.0 * np.minimum(0.0, mp)).sum()
        pa = r["pair_out"].astype(np.float64).reshape(3, 512)
        pen += (pa.sum(1) * np.array([2.0, 3.0, 4.0])).sum()
    pen += _pair_boundary(s_full)
    ce_loss = ce_num / (B * S)
    nnz = float((targets != 0).sum())
    penalty = pen / nnz
    return np.float32(ce_loss + PENALTY_WEIGHT * penalty)


def kernel(logits, targets, predicted_structures, ce_weights):
    in_maps, s, t = make_in_maps(logits, targets, predicted_structures,
                                 ce_weights)
    nc = _get_program()
    res = run_bass_kernel_spmd(nc, in_maps, core_ids=list(range(NCORES)))
    return combine_partials(res.results, s, t, ce_weights)


# revision 23
# speedup vs baseline: 1.5536x; 1.3458x over previous
"""Trainium2 Bass kernel for weighted-CE + structural-penalty loss (V4).

Full inputs -> data-parallel shard over batch across 8 NeuronCores ->
per-core Bass kernel computes small partial sums -> host combines in
float64.

CE = sum_pos w[t]*(lse - x_t):
  A-side: ACT Exp (interleaved), 3-op DVE tree -> se, ACT Ln -> lse,
    fused scalar_tensor_tensor dot (w[t]*lse) with accum_out.
  B-side: no gather at all -- per class c a single fused
    scalar_tensor_tensor (t==c)*x_c with accum_out reads the fp32
    logits directly (one total pass over x); host applies w_c.

Penalty: per row, pen = pair_sum + P_final - 2*min(0, min_prefix(P)),
  P = cumsum((s==1)-(s==2)) via tensor_tensor_scan (fp32 out -- fp16
  out is 6x slower on HW).  Pair products (shifted mask muls) run on
  the otherwise-idle GPSIMD; their sums are 4x-mode tensor_scalar
  accumulates on DVE.  Rows are split into two 2048-halves on
  partitions r | 64+r (host pre-splits, first half has a 3-column real
  halo, second a zero halo); host chains the halves and adds the one
  clamped boundary term.

Host work is restricted to target/weight-derived prep (O(B*S) int
gathers/casts) and tiny per-core partial combination; every operation
touching logits runs on device.
"""

import numpy as np

import concourse.bass as bass
import concourse.mybir as mybir
import concourse.tile as tile
from concourse import bacc
from concourse.bass_utils import run_bass_kernel_spmd

B, S, C = 512, 4096, 8
PENALTY_WEIGHT = 0.1
NCORES = 8
RB = B // NCORES          # rows (batch) per core
N = RB * S                # positions per core
P = 128                   # SBUF partitions
NP = N // P               # positions per partition (2048)
NCH = 4                   # CE processed in NCH free-dim chunks
PCH = NP // NCH           # positions per partition per chunk (512)
SH = S // 2               # penalty half-row length
HALO = 3
SW = SH + HALO

F32 = mybir.dt.float32
F16 = mybir.dt.float16
OP = mybir.AluOpType
AF = mybir.ActivationFunctionType


def _patch_act_tables():
    """Prefer the single table set containing Exp+Ln+Copy so the kernel
    pays one ACT_TABLE_LOAD instead of alternating per chunk."""
    import concourse.hw_specs as hw_specs
    if getattr(hw_specs, "_loss_kernel_tables_patched", False):
        return
    orig = hw_specs.get_activation_tables

    def patched(arch):
        t = orig(arch)
        pref = "natural_log_exp_and_others"
        if pref not in t:
            return t
        return {k: (v if k == pref else set()) for k, v in t.items()}

    hw_specs.get_activation_tables = patched
    bacc.get_activation_tables = patched
    hw_specs._loss_kernel_tables_patched = True


def build_program(compile=True):
    _patch_act_tables()
    nc = bacc.Bacc("TRN2", target_bir_lowering=False, debug=False)

    logits_d = nc.dram_tensor("logits", [P, NP * C], F32, kind="ExternalInput").ap()
    t_d = nc.dram_tensor("t16", [P, NP], F16, kind="ExternalInput").ap()
    wt_d = nc.dram_tensor("wt16", [P, NP], F16, kind="ExternalInput").ap()
    s_d = nc.dram_tensor("s16", [P, SW], F16, kind="ExternalInput").ap()

    ce_d = nc.dram_tensor("ce_acc", [P, NCH], F32, kind="ExternalOutput").ap()
    dx_d = nc.dram_tensor("diag_x", [P, 8, P], F32, kind="ExternalOutput").ap()
    pen_d = nc.dram_tensor("pen_out", [P, 2], F32, kind="ExternalOutput").ap()
    pair_d = nc.dram_tensor("pair_out", [1, 3, 512], F32, kind="ExternalOutput").ap()

    with tile.TileContext(nc) as tc:
        with (
            tc.tile_pool(name="xb", bufs=3) as xb,
            tc.tile_pool(name="eb", bufs=2) as eb,
            tc.tile_pool(name="tre", bufs=2) as tre,
            tc.tile_pool(name="sml", bufs=2) as sml,
            tc.tile_pool(name="stat", bufs=1) as stat,
            tc.tile_pool(name="pen", bufs=1) as pen,
        ):
            t_sb = stat.tile([P, NP], F16)
            wt_sb = stat.tile([P, NP], F16)
            s_sb = pen.tile([P, SW], F16)
            nc.sync.dma_start(out=t_sb, in_=t_d)
            nc.sync.dma_start(out=wt_sb, in_=wt_d)
            nc.sync.dma_start(out=s_sb, in_=s_d)

            ce_sb = stat.tile([P, NCH], F32)

            # masks depend only on t16 -- build them all upfront so the
            # tensor engine can start as soon as each xh chunk lands
            m2a = stat.tile([P, C, NP], F16)
            for c in range(C):
                nc.vector.tensor_scalar(out=m2a[:, c, :], in0=t_sb,
                                        scalar1=float(c), scalar2=None,
                                        op0=OP.is_equal)

            with tc.tile_pool(name="psx", bufs=1, space="PSUM") as psx:
                ps_x = [psx.tile([P, 4, P], F32, name=f"ps_x{q}")
                        for q in range(2)]

                # ---------------- CE chunks ----------------
                for k in range(NCH):
                    fl = k * PCH * C
                    x_t = xb.tile([P, PCH * C], F32, tag="x")
                    nc.sync.dma_start(out=x_t, in_=logits_d[:, fl : fl + PCH * C])
                    x3 = x_t.rearrange("p (n c) -> p n c", c=C)
                    tk = t_sb[:, k * PCH : (k + 1) * PCH]
                    wk = wt_sb[:, k * PCH : (k + 1) * PCH]

                    # B side on TensorE: per-class diagonal matmuls
                    # accumulate S_c partials.
                    xh = eb.tile([P, PCH, C], F16, tag="xh")
                    nc.scalar.activation(xh, x3, AF.Copy)
                    last = k == NCH - 1
                    for c in range(C):
                        q, sl = divmod(c, 4)
                        for b in range(PCH // P):
                            bs = slice(k * PCH + b * P, k * PCH + (b + 1) * P)
                            bl = slice(b * P, (b + 1) * P)
                            nc.tensor.matmul(
                                ps_x[q][:, sl, :], lhsT=m2a[:, c, bs],
                                rhs=xh[:, bl, c],
                                start=(k == 0 and b == 0 and c in (0, 4)),
                                stop=(last and c in (3, 7) and b == PCH // P - 1))

                    # A side: lse then fused dot with w[t]
                    e_t = eb.tile([P, PCH, C], F16, tag="e")
                    nc.scalar.activation(e_t, x3, AF.Exp)
                    s4 = tre.tile([P, PCH, 4], F16, tag="s4")
                    nc.vector.tensor_add(s4, e_t[:, :, 0:4], e_t[:, :, 4:8])
                    s2 = tre.tile([P, PCH, 2], F16, tag="s2")
                    nc.vector.tensor_add(s2, s4[:, :, 0:2], s4[:, :, 2:4])
                    se = sml.tile([P, PCH], F16, tag="se")
                    se3 = se.rearrange("p (n o) -> p n o", o=1)
                    nc.vector.tensor_add(se3, s2[:, :, 0:1], s2[:, :, 1:2])
                    lse = sml.tile([P, PCH], F16, tag="lse")
                    nc.scalar.activation(lse, se, AF.Ln)
                    ja = sml.tile([P, PCH], F16, tag="ja")
                    nc.vector.scalar_tensor_tensor(
                        out=ja, in0=wk, scalar=1.0, in1=lse,
                        op0=OP.mult, op1=OP.mult,
                        accum_out=ce_sb[:, k : k + 1])

                nc.sync.dma_start(out=ce_d, in_=ce_sb)
                dx_sb = stat.tile([P, 8, P], F32)
                for q in range(2):
                    nc.scalar.activation(dx_sb[:, q * 4 : (q + 1) * 4, :],
                                         ps_x[q][:, :, :], AF.Copy)
                nc.sync.dma_start(out=dx_d, in_=dx_sb)

            # ---------------- penalty ----------------
            lp_t = pen.tile([P, SW], F16)
            rp_t = pen.tile([P, SW], F16)
            ee_t = pen.tile([P, SW], F16)
            nc.vector.tensor_scalar(out=lp_t, in0=s_sb, scalar1=1.0,
                                    scalar2=None, op0=OP.is_equal)
            nc.vector.tensor_scalar(out=rp_t, in0=s_sb, scalar1=2.0,
                                    scalar2=None, op0=OP.is_equal)
            nc.vector.tensor_scalar(out=ee_t, in0=s_sb, scalar1=3.0,
                                    scalar2=None, op0=OP.is_equal)

            # shifted pair products on GPSIMD (otherwise idle)
            er_t = pen.tile([P, SW], F16)
            eer_t = pen.tile([P, SW], F16)
            prs = pen.tile([P, 3, SH], F16)
            nc.vector.tensor_mul(er_t[:, 0 : SW - 1], ee_t[:, 0 : SW - 1],
                                 rp_t[:, 1:SW])
            nc.vector.tensor_mul(eer_t[:, 0 : SW - 2], ee_t[:, 0 : SW - 2],
                                 er_t[:, 1 : SW - 1])
            nc.vector.tensor_mul(prs[:, 0, :], lp_t[:, 0:SH],
                                 rp_t[:, 1 : SH + 1])
            nc.vector.tensor_mul(prs[:, 1, :], lp_t[:, 0:SH],
                                 er_t[:, 1 : SH + 1])
            nc.vector.tensor_mul(prs[:, 2, :], lp_t[:, 0:SH],
                                 eer_t[:, 1 : SH + 1])

            # pair sums on the idle tensor engine: ones-matmul column
            # reduction into one PSUM bank per pair kind; host finishes.
            with tc.tile_pool(name="ps", bufs=1, space="PSUM") as psp:
                ones_t = stat.tile([P, 1], F16)
                nc.vector.memset(ones_t, 1.0)
                ps_pair = psp.tile([1, 3, 512], F32, name="ps_pair")
                nmm = SH // 512
                for i in range(3):
                    for wmm in range(nmm):
                        nc.tensor.matmul(
                            ps_pair[:, i, :], lhsT=ones_t,
                            rhs=prs[:, i, wmm * 512 : (wmm + 1) * 512],
                            start=(wmm == 0), stop=(wmm == nmm - 1))
                pair_sb = stat.tile([1, 3, 512], F32)
                nc.scalar.activation(pair_sb, ps_pair, AF.Copy)
                nc.sync.dma_start(out=pair_d, in_=pair_sb)

            pen_sb = stat.tile([P, 2], F32)

            # P = cumsum(lp - rp); need P_final and min-prefix (fp32 out --
            # fp16-out scan measured 6x slower)
            p_t = pen.tile([P, SH], F32)
            nc.vector.tensor_tensor_scan(out=p_t, data0=lp_t[:, 0:SH],
                                         data1=rp_t[:, 0:SH], initial=0.0,
                                         op0=OP.add, op1=OP.subtract)
            nc.vector.tensor_copy(out=pen_sb[:, 0:1], in_=p_t[:, SH - 1 : SH])
            nc.vector.tensor_reduce(out=pen_sb[:, 1:2], in_=p_t,
                                    axis=mybir.AxisListType.X, op=OP.min)
            nc.sync.dma_start(out=pen_d, in_=pen_sb)

    if compile:
        nc.compile()
    return nc


_program = None


def _get_program():
    global _program
    if _program is None:
        _program = build_program()
    return _program


def _pair_boundary(s):
    """The only clamped boundary pair term not covered on device:
    4 * [s[S-3]==1][s[S-2]==3][s[S-1]==2] per row."""
    m = (s[:, -3] == 1) & (s[:, -2] == 3) & (s[:, -1] == 2)
    return 4.0 * float(m.sum())


def make_in_maps(logits, targets, predicted_structures, ce_weights):
    lg = np.ascontiguousarray(logits, dtype=np.float32)
    t = np.asarray(targets)
    s = np.asarray(predicted_structures).reshape(B, S)
    t16 = t.astype(np.float16)
    wt16 = np.asarray(ce_weights, np.float32)[t.astype(np.int64)].astype(np.float16)
    s16 = s.astype(np.float16)
    in_maps = []
    for core in range(NCORES):
        rows = slice(core * RB, (core + 1) * RB)
        sc = s16[rows]                      # [RB, S]
        sh = np.zeros((P, SW), np.float16)
        sh[0:RB, :] = sc[:, 0:SW]           # first half + real halo
        sh[RB:P, 0:SH] = sc[:, SH:S]        # second half, zero halo
        in_maps.append({
            "logits": lg[rows].reshape(P, NP * C),
            "t16": np.ascontiguousarray(t16[rows].reshape(P, NP)),
            "wt16": np.ascontiguousarray(wt16[rows].reshape(P, NP)),
            "s16": sh,
        })
    return in_maps, s, t


def combine_partials(results, s_full, targets, ce_weights):
    """Host-side (float64) combination of per-core device partials."""
    w = np.asarray(ce_weights, np.float64)
    ce_num = 0.0
    pen = 0.0
    p_idx = np.arange(P)
    for r in results:
        ce_num += r["ce_acc"].astype(np.float64).sum()     # sum w[t]*lse
        dx = r["diag_x"].astype(np.float64)                # [P, 8, P]
        Sc = dx[p_idx, :, p_idx].sum(0)
        ce_num -= (Sc * w).sum()                           # sum w_c * S_c
        po = r["pen_out"].astype(np.float64)        # [P, 2]
        pfa, mpa = po[0:RB, 0], po[0:RB, 1]
        pfb, mpb = po[RB:P, 0], po[RB:P, 1]
        pf = pfa + pfb
        mp = np.minimum(mpa, pfa + mpb)
        pen += (pf - 2.0 * np.minimum(0.0, mp)).sum()
        pa = r["pair_out"].astype(np.float64).reshape(3, 512)
        pen += (pa.sum(1) * np.array([2.0, 3.0, 4.0])).sum()
    pen += _pair_boundary(s_full)
    ce_loss = ce_num / (B * S)
    nnz = float((targets != 0).sum())
    penalty = pen / nnz
    return np.float32(ce_loss + PENALTY_WEIGHT * penalty)


def kernel(logits, targets, predicted_structures, ce_weights):
    in_maps, s, t = make_in_maps(logits, targets, predicted_structures,
                                 ce_weights)
    nc = _get_program()
    res = run_bass_kernel_spmd(nc, in_maps, core_ids=list(range(NCORES)))
    return combine_partials(res.results, s, t, ce_weights)


# revision 24
# speedup vs baseline: 1.5574x; 1.0024x over previous
"""Trainium2 Bass kernel for weighted-CE + structural-penalty loss (V4).

Full inputs -> data-parallel shard over batch across 8 NeuronCores ->
per-core Bass kernel computes small partial sums -> host combines in
float64.

CE = sum_pos w[t]*(lse - x_t):
  A-side: ACT Exp (interleaved), 3-op DVE tree -> se, ACT Ln -> lse,
    fused scalar_tensor_tensor dot (w[t]*lse) with accum_out.
  B-side: no gather at all -- per class c a single fused
    scalar_tensor_tensor (t==c)*x_c with accum_out reads the fp32
    logits directly (one total pass over x); host applies w_c.

Penalty: per row, pen = pair_sum + P_final - 2*min(0, min_prefix(P)),
  P = cumsum((s==1)-(s==2)) via tensor_tensor_scan (fp32 out -- fp16
  out is 6x slower on HW).  Pair products (shifted mask muls) run on
  the otherwise-idle GPSIMD; their sums are 4x-mode tensor_scalar
  accumulates on DVE.  Rows are split into two 2048-halves on
  partitions r | 64+r (host pre-splits, first half has a 3-column real
  halo, second a zero halo); host chains the halves and adds the one
  clamped boundary term.

Host work is restricted to target/weight-derived prep (O(B*S) int
gathers/casts) and tiny per-core partial combination; every operation
touching logits runs on device.
"""

import numpy as np

import concourse.bass as bass
import concourse.mybir as mybir
import concourse.tile as tile
from concourse import bacc
from concourse.bass_utils import run_bass_kernel_spmd

B, S, C = 512, 4096, 8
PENALTY_WEIGHT = 0.1
NCORES = 8
RB = B // NCORES          # rows (batch) per core
N = RB * S                # positions per core
P = 128                   # SBUF partitions
NP = N // P               # positions per partition (2048)
NCH = 8                   # CE processed in NCH free-dim chunks
PCH = NP // NCH           # positions per partition per chunk (512)
SH = S // 2               # penalty half-row length
HALO = 3
SW = SH + HALO

F32 = mybir.dt.float32
F16 = mybir.dt.float16
OP = mybir.AluOpType
AF = mybir.ActivationFunctionType


def _patch_act_tables():
    """Prefer the single table set containing Exp+Ln+Copy so the kernel
    pays one ACT_TABLE_LOAD instead of alternating per chunk."""
    import concourse.hw_specs as hw_specs
    if getattr(hw_specs, "_loss_kernel_tables_patched", False):
        return
    orig = hw_specs.get_activation_tables

    def patched(arch):
        t = orig(arch)
        pref = "natural_log_exp_and_others"
        if pref not in t:
            return t
        return {k: (v if k == pref else set()) for k, v in t.items()}

    hw_specs.get_activation_tables = patched
    bacc.get_activation_tables = patched
    hw_specs._loss_kernel_tables_patched = True


def build_program(compile=True):
    _patch_act_tables()
    nc = bacc.Bacc("TRN2", target_bir_lowering=False, debug=False)

    logits_d = nc.dram_tensor("logits", [P, NP * C], F32, kind="ExternalInput").ap()
    t_d = nc.dram_tensor("t16", [P, NP], F16, kind="ExternalInput").ap()
    wt_d = nc.dram_tensor("wt16", [P, NP], F16, kind="ExternalInput").ap()
    s_d = nc.dram_tensor("s16", [P, SW], F16, kind="ExternalInput").ap()

    ce_d = nc.dram_tensor("ce_acc", [P, NCH], F32, kind="ExternalOutput").ap()
    dx_d = nc.dram_tensor("diag_x", [P, 8, P], F32, kind="ExternalOutput").ap()
    pen_d = nc.dram_tensor("pen_out", [P, 2], F32, kind="ExternalOutput").ap()
    pair_d = nc.dram_tensor("pair_out", [1, 3, 512], F32, kind="ExternalOutput").ap()

    with tile.TileContext(nc) as tc:
        with (
            tc.tile_pool(name="xb", bufs=3) as xb,
            tc.tile_pool(name="eb", bufs=2) as eb,
            tc.tile_pool(name="tre", bufs=2) as tre,
            tc.tile_pool(name="sml", bufs=2) as sml,
            tc.tile_pool(name="stat", bufs=1) as stat,
            tc.tile_pool(name="pen", bufs=1) as pen,
        ):
            t_sb = stat.tile([P, NP], F16)
            wt_sb = stat.tile([P, NP], F16)
            s_sb = pen.tile([P, SW], F16)
            nc.sync.dma_start(out=t_sb, in_=t_d)
            nc.sync.dma_start(out=wt_sb, in_=wt_d)
            nc.sync.dma_start(out=s_sb, in_=s_d)

            ce_sb = stat.tile([P, NCH], F32)

            # masks depend only on t16 -- build them all upfront so the
            # tensor engine can start as soon as each xh chunk lands
            m2a = stat.tile([P, C, NP], F16)
            for c in range(C):
                nc.vector.tensor_scalar(out=m2a[:, c, :], in0=t_sb,
                                        scalar1=float(c), scalar2=None,
                                        op0=OP.is_equal)

            # ---------------- penalty ----------------
            lp_t = pen.tile([P, SW], F16)
            rp_t = pen.tile([P, SW], F16)
            ee_t = pen.tile([P, SW], F16)
            nc.vector.tensor_scalar(out=lp_t, in0=s_sb, scalar1=1.0,
                                    scalar2=None, op0=OP.is_equal)
            nc.vector.tensor_scalar(out=rp_t, in0=s_sb, scalar1=2.0,
                                    scalar2=None, op0=OP.is_equal)
            nc.vector.tensor_scalar(out=ee_t, in0=s_sb, scalar1=3.0,
                                    scalar2=None, op0=OP.is_equal)

            # shifted pair products on GPSIMD (otherwise idle)
            er_t = pen.tile([P, SW], F16)
            eer_t = pen.tile([P, SW], F16)
            prs = pen.tile([P, 3, SH], F16)
            nc.vector.tensor_mul(er_t[:, 0 : SW - 1], ee_t[:, 0 : SW - 1],
                                 rp_t[:, 1:SW])
            nc.vector.tensor_mul(eer_t[:, 0 : SW - 2], ee_t[:, 0 : SW - 2],
                                 er_t[:, 1 : SW - 1])
            nc.vector.tensor_mul(prs[:, 0, :], lp_t[:, 0:SH],
                                 rp_t[:, 1 : SH + 1])
            nc.vector.tensor_mul(prs[:, 1, :], lp_t[:, 0:SH],
                                 er_t[:, 1 : SH + 1])
            nc.vector.tensor_mul(prs[:, 2, :], lp_t[:, 0:SH],
                                 eer_t[:, 1 : SH + 1])

            # pair sums on the idle tensor engine: ones-matmul column
            # reduction into one PSUM bank per pair kind; host finishes.
            with tc.tile_pool(name="ps", bufs=1, space="PSUM") as psp:
                ones_t = stat.tile([P, 1], F16)
                nc.vector.memset(ones_t, 1.0)
                ps_pair = psp.tile([1, 3, 512], F32, name="ps_pair")
                nmm = SH // 512
                for i in range(3):
                    for wmm in range(nmm):
                        nc.tensor.matmul(
                            ps_pair[:, i, :], lhsT=ones_t,
                            rhs=prs[:, i, wmm * 512 : (wmm + 1) * 512],
                            start=(wmm == 0), stop=(wmm == nmm - 1))
                pair_sb = stat.tile([1, 3, 512], F32)
                nc.scalar.activation(pair_sb, ps_pair, AF.Copy)
                nc.sync.dma_start(out=pair_d, in_=pair_sb)

            pen_sb = stat.tile([P, 2], F32)

            # P = cumsum(lp - rp); need P_final and min-prefix (fp32 out --
            # fp16-out scan measured 6x slower)
            p_t = pen.tile([P, SH], F32)
            nc.vector.tensor_tensor_scan(out=p_t, data0=lp_t[:, 0:SH],
                                         data1=rp_t[:, 0:SH], initial=0.0,
                                         op0=OP.add, op1=OP.subtract)
            nc.vector.tensor_copy(out=pen_sb[:, 0:1], in_=p_t[:, SH - 1 : SH])
            nc.vector.tensor_reduce(out=pen_sb[:, 1:2], in_=p_t,
                                    axis=mybir.AxisListType.X, op=OP.min)
            nc.sync.dma_start(out=pen_d, in_=pen_sb)

            with tc.tile_pool(name="psx", bufs=1, space="PSUM") as psx:
                ps_x = [psx.tile([P, 4, P], F32, name=f"ps_x{q}")
                        for q in range(2)]

                # ---------------- CE chunks ----------------
                for k in range(NCH):
                    fl = k * PCH * C
                    x_t = xb.tile([P, PCH * C], F32, tag="x")
                    nc.sync.dma_start(out=x_t, in_=logits_d[:, fl : fl + PCH * C])
                    x3 = x_t.rearrange("p (n c) -> p n c", c=C)
                    tk = t_sb[:, k * PCH : (k + 1) * PCH]
                    wk = wt_sb[:, k * PCH : (k + 1) * PCH]

                    # B side on TensorE: per-class diagonal matmuls
                    # accumulate S_c partials.
                    xh = eb.tile([P, PCH, C], F16, tag="xh")
                    nc.scalar.activation(xh, x3, AF.Copy)
                    last = k == NCH - 1
                    for c in range(C):
                        q, sl = divmod(c, 4)
                        for b in range(PCH // P):
                            bs = slice(k * PCH + b * P, k * PCH + (b + 1) * P)
                            bl = slice(b * P, (b + 1) * P)
                            nc.tensor.matmul(
                                ps_x[q][:, sl, :], lhsT=m2a[:, c, bs],
                                rhs=xh[:, bl, c],
                                start=(k == 0 and b == 0 and c in (0, 4)),
                                stop=(last and c in (3, 7) and b == PCH // P - 1))

                    # A side: lse then fused dot with w[t]
                    e_t = eb.tile([P, PCH, C], F16, tag="e")
                    nc.scalar.activation(e_t, x3, AF.Exp)
                    s4 = tre.tile([P, PCH, 4], F16, tag="s4")
                    nc.vector.tensor_add(s4, e_t[:, :, 0:4], e_t[:, :, 4:8])
                    s2 = tre.tile([P, PCH, 2], F16, tag="s2")
                    nc.vector.tensor_add(s2, s4[:, :, 0:2], s4[:, :, 2:4])
                    se = sml.tile([P, PCH], F16, tag="se")
                    se3 = se.rearrange("p (n o) -> p n o", o=1)
                    nc.vector.tensor_add(se3, s2[:, :, 0:1], s2[:, :, 1:2])
                    lse = sml.tile([P, PCH], F16, tag="lse")
                    nc.scalar.activation(lse, se, AF.Ln)
                    ja = sml.tile([P, PCH], F16, tag="ja")
                    nc.vector.scalar_tensor_tensor(
                        out=ja, in0=wk, scalar=1.0, in1=lse,
                        op0=OP.mult, op1=OP.mult,
                        accum_out=ce_sb[:, k : k + 1])

                nc.sync.dma_start(out=ce_d, in_=ce_sb)
                dx_sb = stat.tile([P, 8, P], F32)
                for q in range(2):
                    nc.scalar.activation(dx_sb[:, q * 4 : (q + 1) * 4, :],
                                         ps_x[q][:, :, :], AF.Copy)
                nc.sync.dma_start(out=dx_d, in_=dx_sb)



    if compile:
        nc.compile()
    return nc


_program = None


def _get_program():
    global _program
    if _program is None:
        _program = build_program()
    return _program


def _pair_boundary(s):
    """The only clamped boundary pair term not covered on device:
    4 * [s[S-3]==1][s[S-2]==3][s[S-1]==2] per row."""
    m = (s[:, -3] == 1) & (s[:, -2] == 3) & (s[:, -1] == 2)
    return 4.0 * float(m.sum())


def make_in_maps(logits, targets, predicted_structures, ce_weights):
    lg = np.ascontiguousarray(logits, dtype=np.float32)
    t = np.asarray(targets)
    s = np.asarray(predicted_structures).reshape(B, S)
    t16 = t.astype(np.float16)
    wt16 = np.asarray(ce_weights, np.float32)[t.astype(np.int64)].astype(np.float16)
    s16 = s.astype(np.float16)
    in_maps = []
    for core in range(NCORES):
        rows = slice(core * RB, (core + 1) * RB)
        sc = s16[rows]                      # [RB, S]
        sh = np.zeros((P, SW), np.float16)
        sh[0:RB, :] = sc[:, 0:SW]           # first half + real halo
        sh[RB:P, 0:SH] = sc[:, SH:S]        # second half, zero halo
        in_maps.append({
            "logits": lg[rows].reshape(P, NP * C),
            "t16": np.ascontiguousarray(t16[rows].reshape(P, NP)),
            "wt16": np.ascontiguousarray(wt16[rows].reshape(P, NP)),
            "s16": sh,
        })
    return in_maps, s, t


def combine_partials(results, s_full, targets, ce_weights):
    """Host-side (float64) combination of per-core device partials."""
    w = np.asarray(ce_weights, np.float64)
    ce_num = 0.0
    pen = 0.0
    p_idx = np.arange(P)
    for r in results:
        ce_num += r["ce_acc"].astype(np.float64).sum()     # sum w[t]*lse
        dx = r["diag_x"].astype(np.float64)                # [P, 8, P]
        Sc = dx[p_idx, :, p_idx].sum(0)
        ce_num -= (Sc * w).sum()                           # sum w_c * S_c
        po = r["pen_out"].astype(np.float64)        # [P, 2]
        pfa, mpa = po[0:RB, 0], po[0:RB, 1]
        pfb, mpb = po[RB:P, 0], po[RB:P, 1]
        pf = pfa + pfb
        mp = np.minimum(mpa, pfa + mpb)
        pen += (pf - 2.0 * np.minimum(0.0, mp)).sum()
        pa = r["pair_out"].astype(np.float64).reshape(3, 512)
        pen += (pa.sum(1) * np.array([2.0, 3.0, 4.0])).sum()
    pen += _pair_boundary(s_full)
    ce_loss = ce_num / (B * S)
    nnz = float((targets != 0).sum())
    penalty = pen / nnz
    return np.float32(ce_loss + PENALTY_WEIGHT * penalty)


def kernel(logits, targets, predicted_structures, ce_weights):
    in_maps, s, t = make_in_maps(logits, targets, predicted_structures,
                                 ce_weights)
    nc = _get_program()
    res = run_bass_kernel_spmd(nc, in_maps, core_ids=list(range(NCORES)))
    return combine_partials(res.results, s, t, ce_weights)
